# revision 42
# baseline (speedup 1.0000x reference)
"""SAGAN-style self-attention kernel for Trainium2 (8 NeuronCores, SPMD).

Problem: x[8, 64, 64, 256]; per sample (N=4096 positions, C=256):
    f = x@Wf + bf; g = x@Wg + bg; h = x@Wh + bh
    out = gamma * softmax(g @ f^T) @ h + x

Sharding: data-parallel, one batch sample per NeuronCore (8 cores).

v7 design notes (v5 + TimelineSim-guided pipeline work; the local
timeline model tracks this hardware within ~2%, so it is the tuning
oracle):
  - scores in transposed layout sT[k, q]; constant-shift softmax (shift
    invariance) with the denominator from a ones-column on h; division in
    the epilogue.  All matmuls bf16 (fp8 DoubleRow was evaluated: 2x PE
    rate, but exp->fp8 needs a per-query shift within ~11 nats of the row
    max, row maxes spread +-17 nats around any computable estimate, and
    the fp32->fp8 conversion produces inf, not saturation - unusable).
  - position tile tau holds positions {T*256 + 2p + u}: each SBUF
    partition maps to two consecutive DRAM rows, so x/out DMAs move 2KB
    contiguous lines.
  - prologue: per-group pipeline [x-DMA -> bf16 cast (ACT early groups /
    DVE late) -> bf16 PE transposes (1cyc/row) -> h/f/g projections ->
    woven q-tile-0 QK + exp].  g^T is computed once into strip 0 and
    DMA-replicated to the other row strips (PE columns are the scarce
    resource).  Weights/biases ride the gpsimd DMA queue, x alternates
    sync/gpsimd; every dma_start costs ~0.7-1.1us of issuing-queue time,
    so DMA count and queue order are tuned (single-DMA weight loads via
    strided APs, replicated bias loads via [0,4] partition tiling).
    make_identity is first on the Pool queue; a dummy ACT op up front
    pulls the activation-table load to t~0.
  - main loop: score tiles [128,1024] fp32, 2 sc buffers (4 banks) + 4 AV
    accumulators (4 banks) = all 8 PSUM banks.  exp items [128,1024]
    ACT-paced; AV emitted before the dependent QK(idx+2) so the in-order
    PE queue never head-of-line blocks; epilogue uses one fused
    scalar_tensor_tensor per tile and stores in two DMA halves.
  - hh AND x_sb are allocated outside the per-iteration scope and
    ping-ponged across iterations: tile-granularity aliasing otherwise
    makes iter k+1's x-load/h-proj wait for iter k's LAST reader (the
    qt7 epilogue / final AV), serializing the iteration boundary
    (x_sb ping-pong alone recovered ~8us/iter steady-state).
  - h-projection PSUM drains as one strided pair-copy per psum buffer
    (dest skips the ones column); late groups put one pair on ACT since
    DVE paces the late prologue.
  - model accounting per core: PE busy ~182us (AV 110, QK 55, proj 17),
    ACT ~137us, steady-state ~194us/iter.  PE stream columns are the
    floor; 1 col/cycle at 2.4GHz for bf16 regardless of contraction size
    (tile_position packing does not overlap streams on this hardware).
"""

import os
import numpy as np
from contextlib import ExitStack

import concourse.bass as bass
import concourse.tile as tile
from concourse import bacc, mybir
from concourse.bass_utils import run_bass_kernel_spmd
from concourse.bass_interp import get_hw_module
from concourse.masks import make_identity

F32 = mybir.dt.float32
BF16 = mybir.dt.bfloat16
AF = mybir.ActivationFunctionType

N_CORES = 8
N = 4096          # positions per sample (64*64)
C = 256           # channels
D = 32            # f/g projection dim
NT = N // 128     # 32 position tiles of 128
QT = N // 512     # 8 query tiles of 512
NH = 16           # half-key-groups (2 k-chunks) per q-tile


def _attention_kernel(ctx: ExitStack, tc: tile.TileContext, out_ap, x_ap, kf_ap,
                      kg_ap, kh_ap, bf_ap, bg_ap, bh_ap, gamma_ap, hh=None,
                      x_sb=None):
    nc = tc.nc

    persist = ctx.enter_context(tc.tile_pool(name="persist", bufs=1))

    # ---- persistent SBUF tensors -------------------------------------
    if x_sb is None:
        # residual, tau=(T,u); callers running back-to-back iterations
        # pass an iteration-alternating tile: otherwise iter k+1's x-load
        # DMA waits for iter k's LAST x_sb reader (the qt7 epilogue),
        # serializing the iteration boundary
        x_sb = persist.tile([128, NT // 2, 2, C], F32)
    xT = persist.tile([128, 2, N], BF16)            # x^T, c-chunk major
    fTp = persist.tile([128, QT * 128], BF16)       # f^T packed into 4 row strips
    gTr = persist.tile([128, N], BF16)              # g^T replicated in 4 row strips
    if hh is None:
        # h chunks [k, c] + ones column; callers running back-to-back
        # iterations pass an iteration-alternating tile instead so that
        # iter k+1's h-projection does not serialize behind iter k's
        # last AV read of hh
        hh = persist.tile([128, NT, C + 1], BF16)
    wf = persist.tile([128, 2, D], BF16)
    wg = persist.tile([128, 2, D], BF16)
    wh = persist.tile([128, 2, C], BF16)
    bias_f_rep = persist.tile([128, 1], F32)
    bias_g_rep = persist.tile([128, 1], F32)
    gb_row = persist.tile([128, C], F32)            # gamma * bias_h
    gamma_rep = persist.tile([128, 1], F32)
    ident_b = persist.tile([128, 128], BF16)
    shift = persist.tile([128, 1], F32)
    ex0 = persist.tile([128, NH * 1024], BF16)      # even-qt exp tiles (weave fills qt0)
    ex1 = persist.tile([128, NH * 1024], BF16)      # odd-qt exp tiles

    # 2-row interleave: DRAM row T*256 + 2p + u <-> x_sb[p, T, u, :]
    x2 = x_ap.rearrange("(T p u) c -> p T u c", p=128, u=2)
    out2 = out_ap.rearrange("(T p u) c -> p T u c", p=128, u=2)

    work = ctx.enter_context(tc.tile_pool(name="work", bufs=2))
    outb = ctx.enter_context(tc.tile_pool(name="outb", bufs=2))

    def qk(qt, h, ps):
        # sT[k, q] for 2 k-chunks; chunk streams land in separate PSUM banks
        for i in range(2):
            kc = h * 2 + i
            g = kc % 4
            nc.tensor.matmul(
                ps[:, 512 * i:512 * (i + 1)],
                lhsT=fTp[32 * g:32 * (g + 1),
                         (kc // 4) * 128:(kc // 4 + 1) * 128],
                rhs=gTr[32 * g:32 * (g + 1), qt * 512:(qt + 1) * 512],
                start=True, stop=True,
                tile_position=(32 * g, 0))

    def av(po, h, ex, base=0):
        for i in range(2):
            kc = h * 2 + i
            for j in range(4):
                o = base + 512 * i + 128 * j
                nc.tensor.matmul(
                    po[j][:],
                    lhsT=ex[:, o:o + 128],
                    rhs=hh[:, kc, :],
                    start=(kc == 0), stop=(kc == NT - 1))

    def fold_gb(qt):
        for j in range(4):
            t = qt * 4 + j
            nc.vector.tensor_add(x_sb[:, t // 2, t % 2, :],
                                 x_sb[:, t // 2, t % 2, :], gb_row[:])

    def epilogue(qt, po):
        # out = (gamma/sumexp) * o + (x + gamma*bias_h); the out-DMA goes
        # in two halves so the last tile's store overlaps the second half
        ot = outb.tile([128, 2, 2, C], F32, tag="ot", name="ot")
        eng = nc.sync if qt % 2 == 0 else nc.scalar
        for j in range(4):
            t = qt * 4 + j
            r = work.tile([128, 1], F32, tag="r", name="r")
            nc.vector.reciprocal(r[:], po[j][:, C:C + 1])
            rg = work.tile([128, 1], F32, tag="rg", name="rg")
            nc.vector.tensor_mul(rg[:], r[:], gamma_rep[:])
            # fused (po * rg) + x in one DVE op
            nc.vector.scalar_tensor_tensor(
                out=ot[:, j // 2, j % 2, :], in0=po[j][:, 0:C],
                scalar=rg[:, 0:1], in1=x_sb[:, t // 2, t % 2, :],
                op0=mybir.AluOpType.mult, op1=mybir.AluOpType.add)
            if j == 1:
                eng.dma_start(out=out2[:, 2 * qt:2 * qt + 1, :, :],
                              in_=ot[:, 0:1, :, :])
        eng.dma_start(out=out2[:, 2 * qt + 1:2 * qt + 2, :, :],
                      in_=ot[:, 1:2, :, :])

    with tc.tile_pool(name="pro_w", bufs=1) as pro_w, \
         tc.tile_pool(name="pro_psum", bufs=2, space="PSUM") as pro_psum, \
         tc.tile_pool(name="wsc", bufs=1, space="PSUM") as wsc_pool, \
         tc.tile_pool(name="pro_tmp", bufs=3) as pro_tmp:

        # ---- constants / weights FIRST (small; unblocks group-0 work
        # immediately instead of queueing 4MB of x ahead of them) -------
        # identity FIRST on the Pool queue — the first PE transposes need
        # it, and every DMA-issue on Pool costs ~1us of queue time
        make_identity(nc, ident_b[:])
        # tiny ACT op up front so the activation table load happens at t~0
        # instead of delaying the first real cast
        warm = pro_w.tile([128, 1], F32)
        nc.vector.memset(warm[:], 0.0)
        nc.scalar.copy(warm[:], warm[:])

        # weights/biases ride the gpsimd queue so the sync queue's head is
        # free for x group 0 (PE's first dependency); group 0 is split in
        # two so the first transposes can start after ~256KB
        wf32 = pro_w.tile([128, 2, D], F32)
        wg32 = pro_w.tile([128, 2, D], F32)
        wh32 = pro_w.tile([128, 2, C], F32)
        # SP queue: x group 0 (split), g1, g2, biases, g4, g6 — ordered by
        # when each is first consumed; every DMA issue costs ~0.7-1.1us of
        # queue time, so order matters more than locality
        nc.sync.dma_start(out=x_sb[:, 0:1, :, :], in_=x2[:, 0:1, :, :])
        nc.sync.dma_start(out=x_sb[:, 1:2, :, :], in_=x2[:, 1:2, :, :])
        nc.sync.dma_start(out=x_sb[:, 2:4, :, :], in_=x2[:, 2:4, :, :])
        nc.sync.dma_start(out=x_sb[:, 4:6, :, :], in_=x2[:, 4:6, :, :])
        nc.sync.dma_start(
            out=bias_f_rep[:],
            in_=bass.AP(tensor=bf_ap.tensor, offset=bf_ap.offset,
                        ap=[[0, 4], [1, 32], [0, 1]]))
        nc.sync.dma_start(
            out=bias_g_rep[:],
            in_=bass.AP(tensor=bg_ap.tensor, offset=bg_ap.offset,
                        ap=[[0, 4], [1, 32], [0, 1]]))
        nc.sync.dma_start(out=x_sb[:, 8:10, :, :], in_=x2[:, 8:10, :, :])
        nc.sync.dma_start(out=x_sb[:, 12:14, :, :], in_=x2[:, 12:14, :, :])

        # Pool queue: weights, x3/x5/x7, bias_h/gamma
        nc.gpsimd.dma_start(
            out=wh32[:],
            in_=bass.AP(tensor=kh_ap.tensor, offset=kh_ap.offset,
                        ap=[[C, 128], [128 * C, 2], [1, C]]))
        nc.gpsimd.dma_start(
            out=wf32[:],
            in_=bass.AP(tensor=kf_ap.tensor, offset=kf_ap.offset,
                        ap=[[D, 128], [128 * D, 2], [1, D]]))
        nc.gpsimd.dma_start(
            out=wg32[:],
            in_=bass.AP(tensor=kg_ap.tensor, offset=kg_ap.offset,
                        ap=[[D, 128], [128 * D, 2], [1, D]]))
        nc.gpsimd.dma_start(out=x_sb[:, 6:8, :, :], in_=x2[:, 6:8, :, :])
        nc.gpsimd.dma_start(out=x_sb[:, 10:12, :, :], in_=x2[:, 10:12, :, :])
        nc.gpsimd.dma_start(out=x_sb[:, 14:16, :, :], in_=x2[:, 14:16, :, :])
        bh_b = bass.AP(tensor=bh_ap.tensor, offset=bh_ap.offset,
                       ap=[[0, 128]] + list(bh_ap.ap))
        bias_row = pro_w.tile([128, C], F32)
        nc.gpsimd.dma_start(out=bias_row[:], in_=bh_b)
        gamma_b = bass.AP(tensor=gamma_ap.tensor, offset=gamma_ap.offset,
                          ap=[[0, 128]] + list(gamma_ap.ap))
        nc.gpsimd.dma_start(out=gamma_rep[:], in_=gamma_b)

        nc.vector.tensor_copy(wf[:], wf32[:])
        nc.vector.tensor_copy(wg[:], wg32[:])
        nc.vector.tensor_copy(wh[:], wh32[:])
        nc.vector.tensor_scalar_mul(gb_row[:], bias_row[:], gamma_rep[:, 0:1])

        # only the ones-column of hh needs initializing
        nc.gpsimd.memset(hh[:, :, C:C + 1], 1.0)
        # scores land in ~[-90, 90]; shift keeps exp inside fp32/bf16 range
        nc.vector.memset(shift[:], -36.0)

        # ---- per-group projections with q-tile-0 QK/exp woven in -----
        for g in range(8):
            # stage 1: cast the group to bf16 (DVE), then 8 bf16 transposes
            # (1 PE cycle/row instead of fp32's 2) into one 1-bank psum
            # tile, then a single wide DVE copy into xT
            xb = pro_tmp.tile([128, 2, 2, C], BF16, tag="xb", name=f"xb{g}")
            # casts on ACT early (DVE busy with copies), DVE for later
            # groups (ACT gets saturated by the woven exps)
            ceng = nc.scalar.copy if g < 4 else nc.vector.tensor_copy
            ceng(xb[:, 0, :, :], x_sb[:, 2 * g, :, :])
            ceng(xb[:, 1, :, :], x_sb[:, 2 * g + 1, :, :])
            ps_tr = pro_psum.tile([128, 2, 512], BF16, tag="tr",
                                  name="ps_tr")
            for c in range(2):
                for ti in range(4):
                    nc.tensor.transpose(
                        ps_tr[:, c, ti * 128:(ti + 1) * 128],
                        xb[:, ti // 2, ti % 2, c * 128:(c + 1) * 128],
                        ident_b[:])
            nc.vector.tensor_copy(xT[:, :, g * 512:(g + 1) * 512], ps_tr[:])
            # stage 2: h-projections, 2 tiles per psum buffer; each pair is
            # drained by ONE strided copy (dest skips the ones column).
            # Late groups put one pair on ACT: DVE is the late-prologue
            # pace-setter (xT copies + casts + bias adds)
            for pair in range(2):
                ps_h = pro_psum.tile([128, 512], F32, tag="big", name="ps_h")
                for ti in range(2):
                    t = g * 4 + pair * 2 + ti
                    for c in range(2):
                        nc.tensor.matmul(ps_h[:, ti * 256:ti * 256 + C],
                                         lhsT=xT[:, c, t * 128:(t + 1) * 128],
                                         rhs=wh[:, c, :],
                                         start=(c == 0), stop=(c == 1))
                t = g * 4 + pair * 2
                dst = hh[:, t:t + 2, 0:C]
                if g >= 4 and pair == 0:
                    nc.scalar.copy(dst, ps_h[:].rearrange("p (two c) -> p two c", two=2))
                else:
                    nc.vector.tensor_copy(dst, ps_h[:].rearrange("p (two c) -> p two c", two=2))

            # f^T packed: strip i <- k-chunk 4g+i
            ps_f = pro_psum.tile([128, 512], F32, tag="big", name="ps_f")
            for i in range(4):
                for c in range(2):
                    nc.tensor.matmul(
                        ps_f[32 * i:32 * (i + 1), 0:128],
                        lhsT=wf[:, c, :],
                        rhs=xT[:, c, (g * 4 + i) * 128:(g * 4 + i + 1) * 128],
                        start=(c == 0), stop=(c == 1),
                        tile_position=(0, 32 * i))
            nc.vector.tensor_scalar_add(fTp[:, g * 128:(g + 1) * 128],
                                        ps_f[:, 0:128], bias_f_rep[:, 0:1])

            # g^T computed ONCE into strip 0, then DMA-replicated to the
            # other 3 row strips (PE columns are the scarce resource; the
            # DMA engine is idle)
            ps_g = pro_psum.tile([128, 512], F32, tag="big", name="ps_g")
            for c in range(2):
                nc.tensor.matmul(ps_g[0:32, :],
                                 lhsT=wg[:, c, :],
                                 rhs=xT[:, c, g * 512:(g + 1) * 512],
                                 start=(c == 0), stop=(c == 1))
            nc.vector.tensor_scalar_add(gTr[0:32, g * 512:(g + 1) * 512],
                                        ps_g[0:32, :], bias_g_rep[0:32, 0:1])
            if g == 0:
                # group 0's strips feed every weave; replicate them now.
                # groups 1..7 are replicated in one shot after group 7.
                for i in range(1, 4):
                    nc.sync.dma_start(out=gTr[32 * i:32 * (i + 1), 0:512],
                                      in_=gTr[0:32, 0:512])
            elif g == 7:
                for i in range(1, 4):
                    nc.sync.dma_start(out=gTr[32 * i:32 * (i + 1), 512:N],
                                      in_=gTr[0:32, 512:N])

            # weave q-tile 0, chunks 4g..4g+3: one 4-bank score tile
            # (po accumulators not yet allocated), single [128,2048] exp
            ps0 = wsc_pool.tile([128, 2048], F32, tag="s2", name="ps0")
            for i in range(4):
                nc.tensor.matmul(
                    ps0[:, 512 * i:512 * (i + 1)],
                    lhsT=fTp[32 * i:32 * (i + 1), g * 128:(g + 1) * 128],
                    rhs=gTr[32 * i:32 * (i + 1), 0:512],
                    start=True, stop=True,
                    tile_position=(32 * i, 0))
            nc.scalar.activation(out=ex0[:, g * 2048:(g + 1) * 2048],
                                 in_=ps0[:], func=AF.Exp, bias=shift[:, 0:1])

    # ---- main attention loop: one continuous pipeline ----------------
    # Phase qt runs AV(qt) on PE while ACT computes ALL of qt+1's exps into
    # the ping-pong buffer (ex0/ex1 by parity); the QK stream (lookahead 2)
    # is continuous across q-tile boundaries, so ACT never waits on PE.
    with tc.tile_pool(name="ps_o", bufs=1, space="PSUM") as ps_o_pool, \
         tc.tile_pool(name="sc", bufs=2, space="PSUM") as sc_pool:

        NSEQ = (QT - 1) * NH        # QK/exp work items for q-tiles 1..7
        pses = {}

        def issue_qk(idx):
            if idx < NSEQ:
                pses[idx] = sc_pool.tile([128, 1024], F32, tag="s",
                                         name=f"s{idx}")
                qk(1 + idx // NH, idx % NH, pses[idx])

        issue_qk(0)
        issue_qk(1)

        for qt in range(QT):
            fold_gb(qt)
            po = [ps_o_pool.tile([128, C + 1], F32, tag=f"o{j}", name=f"po{j}")
                  for j in range(4)]
            exbuf = ex0 if qt % 2 == 0 else ex1
            nbuf = ex1 if qt % 2 == 0 else ex0
            for h in range(NH):
                idx = qt * NH + h           # exp work item for q-tile qt+1
                if idx < NSEQ:
                    nc.scalar.activation(
                        out=nbuf[:, (idx % NH) * 1024:(idx % NH + 1) * 1024],
                        in_=pses.pop(idx)[:], func=AF.Exp, bias=shift[:, 0:1])
                av(po, h, exbuf, base=h * 1024)
                # QK(idx+2) waits on exp(idx) freeing its buffer; emit it
                # AFTER the ready AV work so the in-order PE queue never
                # head-of-line blocks on it.
                if idx < NSEQ:
                    issue_qk(idx + 2)
            epilogue(qt, po)


_PROGRAMS = {}


def _build_program(repeat=1):
    """repeat>1 unrolls the whole kernel body multiple times in one program
    (timing-only: lets host-side wall clocks resolve per-iteration HW time).
    repeat=0 builds a near-empty program to measure fixed dispatch overhead."""
    if repeat in _PROGRAMS:
        return _PROGRAMS[repeat]
    nc = bacc.Bacc("TRN2", target_bir_lowering=False, debug=False,
                   enable_asserts=False, num_devices=N_CORES)
    x_ap = nc.dram_tensor("x", [N, C], F32, kind="ExternalInput").ap()
    kf_ap = nc.dram_tensor("kernel_f", [C, D], F32, kind="ExternalInput").ap()
    kg_ap = nc.dram_tensor("kernel_g", [C, D], F32, kind="ExternalInput").ap()
    kh_ap = nc.dram_tensor("kernel_h", [C, C], F32, kind="ExternalInput").ap()
    bf_ap = nc.dram_tensor("bias_f", [D], F32, kind="ExternalInput").ap()
    bg_ap = nc.dram_tensor("bias_g", [D], F32, kind="ExternalInput").ap()
    bh_ap = nc.dram_tensor("bias_h", [C], F32, kind="ExternalInput").ap()
    gamma_ap = nc.dram_tensor("gamma", [1], F32, kind="ExternalInput").ap()
    out_ap = nc.dram_tensor("out", [N, C], F32, kind="ExternalOutput").ap()

    with tile.TileContext(nc) as tc:
        if repeat == 0:
            with ExitStack() as ctx:
                pool = ctx.enter_context(tc.tile_pool(name="p0", bufs=1))
                t = pool.tile([128, C], F32)
                nc.sync.dma_start(out=t[:], in_=x_ap[0:128, :])
                nc.sync.dma_start(out=out_ap[0:128, :], in_=t[:])
        if repeat > 0:
            with tc.tile_pool(name="hh2", bufs=1) as hh_pool:
                nb = min(repeat, 2)
                hhs = [hh_pool.tile([128, NT, C + 1], BF16, name=f"hh{i}")
                       for i in range(nb)]
                xsbs = [hh_pool.tile([128, NT // 2, 2, C], F32, name=f"xsb{i}")
                        for i in range(nb)]
                for i in range(repeat):
                    with ExitStack() as ctx:
                        _attention_kernel(ctx, tc, out_ap, x_ap, kf_ap, kg_ap,
                                          kh_ap, bf_ap, bg_ap, bh_ap, gamma_ap,
                                          hh=hhs[i % nb], x_sb=xsbs[i % nb])
    nc.compile()
    nc.m = get_hw_module(nc.m)
    _PROGRAMS[repeat] = nc
    return nc


def _make_in_maps(inputs):
    x = np.ascontiguousarray(np.asarray(inputs["x"], dtype=np.float32))
    B = x.shape[0]
    assert x.shape == (B, 64, 64, C) and B == N_CORES
    shared = {
        "kernel_f": np.ascontiguousarray(np.asarray(inputs["kernel_f"], np.float32)),
        "kernel_g": np.ascontiguousarray(np.asarray(inputs["kernel_g"], np.float32)),
        "kernel_h": np.ascontiguousarray(np.asarray(inputs["kernel_h"], np.float32)),
        "bias_f": np.ascontiguousarray(np.asarray(inputs["bias_f"], np.float32)),
        "bias_g": np.ascontiguousarray(np.asarray(inputs["bias_g"], np.float32)),
        "bias_h": np.ascontiguousarray(np.asarray(inputs["bias_h"], np.float32)),
        "gamma": np.ascontiguousarray(np.asarray(inputs["gamma"], np.float32)),
    }
    return [{"x": x[b].reshape(N, C), **shared} for b in range(N_CORES)]


def run(inputs, trace=False, **kw):
    nc = _build_program()
    res = run_bass_kernel_spmd(nc, _make_in_maps(inputs),
                               core_ids=list(range(N_CORES)), trace=trace, **kw)
    out = np.stack([res.results[i]["out"] for i in range(N_CORES)])
    return out.reshape(N_CORES, 64, 64, C).astype(np.float32), res


def kernel(**inputs):
    out, _ = run(inputs)
    return out

